# revision 1
# baseline (speedup 1.0000x reference)
"""Trainium2 Bass kernel for the 27092653703365 contrastive loss.

Strategy (memory-bound; ~138 MB of image features dominates):
  - Data-parallel shard of the batch dim (bs=256) across 8 NeuronCores
    (32 images per core); random_text_features replicated.
  - Per core: stream the [32, 256, 512] image block through SBUF once.
    Each (image, text-row) needs only its dot with one text vector and
    its squared norm, so the stream is elementwise work: DVE does the
    dots (scalar_tensor_tensor + accumulate) and the squares are split
    DVE/ACT so both engines stay under the HBM DMA roofline.
  - Tail rows (a=256..263) vs false texts are processed early so their
    compute fills the DMA ramp; img groups alternate between the SP
    HWDGE ring (f32) and the Pool SWDGE ring (bf16 cast) to split the
    transfer load; all activations stay on one ACT table set (1/sqrt
    computed as exp(-0.5 ln)).
  - No on-device collective: an 8-byte AllGather alone measures ~140us
    on this runtime (trigger/rendezvous dominated), so each core returns
    its 257-float partial (column sums of exp(logits) for its images +
    the row-CE partial) and kernel() finishes the scalar loss on the
    host while unsharding -- a ~2KB numpy epilogue.
"""

import sys

sys.path.insert(0, "/opt/trn_rl_repo")

from contextlib import ExitStack

import numpy as np

import concourse.bass as bass
import concourse.tile as tile
from concourse import mybir
from concourse.bass_utils import run_bass_kernel_spmd

F32 = mybir.dt.float32
BF16 = mybir.dt.bfloat16
AF = mybir.ActivationFunctionType
ALU = mybir.AluOpType
AX = mybir.AxisListType

NCORES = 8
BS, FTN, D = 256, 8, 512
ATN = BS + FTN  # 264
BPC = BS // NCORES  # 32 images per core
# image-group sizes: small first group shortens the DMA ramp; small last
# group lets chunk 0 finish (and its post-processing start) early
GROUPS = [4, 8, 8, 8, 4]
assert sum(GROUPS) == BPC


def _cap_sync_waits(nc: bass.Bass, max_waits: int = 1) -> None:
    """The walrus build in this container encodes at most one sync-wait
    command per instruction ("Too many sync wait commands" in codegen
    otherwise), but Tile freely attaches several. Splitting the surplus
    waits onto single-wait Drain carriers right before the instruction is
    semantically identical: the engine blocks on each in turn.
    """
    for func in nc.m.functions:
        for bb in func.blocks:
            out = []
            for ins in bb.instructions:
                si = ins.sync_info
                if si is not None and len(si.on_wait) > max_waits:
                    waits = list(si.on_wait)
                    extra, keep = waits[:-max_waits], waits[-max_waits:]
                    for k, w in enumerate(extra):
                        d = mybir.InstDrain(
                            name=f"{ins.name}_w{k}",
                            ins=[],
                            outs=[],
                            engine=ins.engine,
                        )
                        d.sync_info = mybir.SyncInfo(on_wait=[w], on_update=[])
                        nc.register_instruction(d, overwrite=True)
                        out.append(d)
                    ins.sync_info = mybir.SyncInfo(
                        on_wait=keep, on_update=list(si.on_update)
                    )
                out.append(ins)
            bb.instructions = out


def build_nc() -> bass.Bass:
    nc = bass.Bass(num_devices=NCORES)

    img = nc.declare_dram_parameter("img", [BPC, ATN, D], F32, isOutput=False)
    rand = nc.declare_dram_parameter("rand", [BS, D], F32, isOutput=False)
    falset = nc.declare_dram_parameter("falset", [BPC * FTN, D], F32, isOutput=False)
    lscale = nc.declare_dram_parameter("lscale", [1], F32, isOutput=False)
    ident = nc.declare_dram_parameter("ident", [128, 128], F32, isOutput=False)
    dmask = nc.declare_dram_parameter("dmask", [128, 2 * BPC], F32, isOutput=False)
    part_out = nc.declare_dram_parameter("part_out", [1, 2 * 128 + 1], F32, isOutput=True)

    with tile.TileContext(nc) as tc, ExitStack() as ctx:
        singles = ctx.enter_context(tc.tile_pool(name="singles", bufs=1))
        imgpool = ctx.enter_context(tc.tile_pool(name="img", bufs=2))
        tmppool = ctx.enter_context(tc.tile_pool(name="tmp", bufs=2))
        small = ctx.enter_context(tc.tile_pool(name="small", bufs=2))
        psum = ctx.enter_context(tc.tile_pool(name="psum", bufs=2, space="PSUM"))
        dram = ctx.enter_context(tc.tile_pool(name="dram", bufs=1, space="DRAM"))

        # ---- preloads (ACT HWDGE ring; img stream owns the SP ring) ---------
        ls_raw = singles.tile([128, 1], F32)
        nc.scalar.dma_start(out=ls_raw, in_=lscale[:].to_broadcast([128, 1]))
        # rand text, a-chunked: rand2[p, c, d] = rand[c*128+p, d]
        # (bf16 via SWDGE cast-DMA halves SBUF-side DMA bytes)
        rand2 = singles.tile([128, 2, D], BF16)
        nc.gpsimd.dma_start(out=rand2, in_=rand[:, :].rearrange("(c p) d -> p c d", p=128))
        # 128x128 identity for PE transposes
        id128 = singles.tile([128, 128], F32)
        nc.scalar.dma_start(out=id128, in_=ident[:, :])
        # one-hot mask of this core's diagonal logits in column layout
        dmk = singles.tile([128, 2, BPC], F32)
        nc.scalar.dma_start(
            out=dmk, in_=dmask[:, :].rearrange("p (c b) -> p c b", c=2)
        )

        scale_b = singles.tile([128, 1], F32)
        nc.scalar.activation(scale_b, ls_raw, AF.Exp)
        ones128 = singles.tile([128, 1], F32)
        nc.vector.memset(ones128, 1.0)
        neg2 = singles.tile([128, 1], F32)
        nc.vector.memset(neg2, -2.0)

        # accumulators
        dots01 = singles.tile([128, 2, BPC], F32)
        nsq01 = singles.tile([128, 2, BPC], F32)

        # rand norms (ACT is free while the first img DMA streams)
        rn_sq = small.tile([128, 2], F32)
        for c in range(2):
            sqr = tmppool.tile([128, D], F32, tag="sqr")
            nc.scalar.activation(
                sqr, rand2[:, c, :], AF.Square, accum_out=rn_sq[:, c : c + 1]
            )
        rn_isc = small.tile([128, 2], F32)
        nc.scalar.activation(rn_isc, rn_sq, AF.Ln)
        nc.scalar.activation(rn_isc, rn_isc, AF.Exp, scale=-0.5)
        nc.vector.tensor_scalar_mul(rn_isc, rn_isc, scale_b)

        # persistent logits state (written chunk by chunk)
        inv01 = singles.tile([128, 2, BPC], F32)
        LB = singles.tile([128, 2, BPC], F32)
        expLB = singles.tile([128, 2, BPC], F32)
        cs = singles.tile([128, 2], F32)
        rs = singles.tile([BPC, 1], F32)

        # ---- main stream (chunk-major): dots + squared norms ----------------
        # Chunk 0 (texts a<128) streams first; its logits post-processing then
        # hides under chunk 1's stream.
        #
        # Engine split across the 152 elementwise units (cost model: DVE STT
        # ~0.57us, ACT Square ~0.87us; GPSIMD has no STT opcode on real HW):
        # DVE takes all 72 dots + ~20 squares, ACT ~60 squares -> ~54us each.
        # square_engine per image slot within an 8-slot block
        SQ_ENG = ["A", "A", "D", "A", "A", "D", "A", "A"]

        def dve_stt(in0, in1, acc):
            o = tmppool.tile([128, D], BF16, tag="sqd")
            nc.vector.scalar_tensor_tensor(
                out=o, in0=in0, scalar=1.0, in1=in1,
                op0=ALU.mult, op1=ALU.mult, accum_out=acc,
            )

        # alternate img groups between the SP HWDGE ring (f32) and the Pool
        # SWDGE ring (bf16 cast): splits the transfer load across both DMA
        # paths and doubles the effective prefetch depth
        grp_ctr = [0]

        def do_cgroup(c, b0, gsz):
            src = img[b0 : b0 + gsz, c * 128 : (c + 1) * 128, :].rearrange(
                "g p d -> p g d"
            )
            if grp_ctr[0] % 2 == 0:
                img_t = imgpool.tile([128, gsz, D], F32, tag=f"imgf{gsz}")
                nc.sync.dma_start(out=img_t, in_=src)
            else:
                img_t = imgpool.tile([128, gsz, D], BF16, tag=f"imgb{gsz}")
                nc.gpsimd.dma_start(out=img_t, in_=src)
            grp_ctr[0] += 1
            for i in range(gsz):
                b = b0 + i
                se = SQ_ENG[b % 8]
                ia = img_t[:, i, :]
                ra = rand2[:, c, :]
                dacc = dots01[:, c, b : b + 1]
                sacc = nsq01[:, c, b : b + 1]
                dve_stt(ia, ra, dacc)
                if se == "A":
                    sqa = tmppool.tile([128, D], BF16, tag="sqa")
                    nc.scalar.activation(sqa, ia, AF.Square, accum_out=sacc)
                else:
                    dve_stt(ia, ia, sacc)

        dcol = singles.tile([128, 2], F32)

        def post_chunk(c):
            nc.scalar.activation(inv01[:, c, :], nsq01[:, c, :], AF.Ln)
            nc.scalar.activation(inv01[:, c, :], inv01[:, c, :], AF.Exp, scale=-0.5)
            nc.vector.tensor_mul(LB[:, c, :], dots01[:, c, :], inv01[:, c, :])
            nc.vector.tensor_scalar_mul(
                LB[:, c, :], LB[:, c, :], rn_isc[:, c : c + 1]
            )
            nc.scalar.activation(expLB[:, c, :], LB[:, c, :], AF.Exp)
            nc.vector.tensor_reduce(
                cs[:, c : c + 1], expLB[:, c, :], axis=AX.X, op=ALU.add
            )
            # this chunk's share of the diagonal partial
            dprod = tmppool.tile([128, BPC], F32, tag="dprod")
            nc.vector.scalar_tensor_tensor(
                out=dprod,
                in0=LB[:, c, :],
                scalar=1.0,
                in1=dmk[:, c, :],
                op0=ALU.mult,
                op1=ALU.mult,
                accum_out=dcol[:, c : c + 1],
            )
            pt = psum.tile([BPC, 128], F32, tag="ptr")
            nc.tensor.transpose(pt, expLB[:, c, :], id128)
            rc = small.tile([BPC, 1], F32, tag="rc")
            nc.vector.tensor_reduce(rc, pt, axis=AX.X, op=ALU.add)
            if c == 0:
                nc.vector.tensor_add(rs, rc, rst)
            else:
                nc.vector.tensor_add(rs, rs, rc)

        # first (small) group before the tail block: compute starts ASAP
        do_cgroup(0, 0, GROUPS[0])

        # false texts + image tail rows, both [b=32, f=8, d] (after the first
        # img group in the SWDGE queue so dots start ASAP)
        false_t = singles.tile([BPC, FTN, D], BF16)
        nc.gpsimd.dma_start(
            out=false_t, in_=falset[:, :].rearrange("(b f) d -> b f d", f=FTN)
        )
        tail_t = singles.tile([BPC, FTN, D], BF16)
        nc.gpsimd.dma_start(out=tail_t, in_=img[:, BS:ATN, :])

        # ---- tail rows vs false texts (overlaps the img stream) -------------
        ltr = small.tile([BPC, FTN], F32)
        nsq_t = small.tile([BPC, FTN], F32)
        nsq_f = small.tile([BPC, FTN], F32)
        tsq_unit = 0
        for f in range(FTN):
            prodf = tmppool.tile([BPC, D], BF16, tag="prodf")
            nc.vector.scalar_tensor_tensor(
                out=prodf,
                in0=tail_t[:, f, :],
                scalar=1.0,
                in1=false_t[:, f, :],
                op0=ALU.mult,
                op1=ALU.mult,
                accum_out=ltr[:, f : f + 1],
            )
            for src_t, acc in ((tail_t, nsq_t), (false_t, nsq_f)):
                if tsq_unit % 4 != 3:
                    sq = tmppool.tile([BPC, D], BF16, tag="tsqa")
                    nc.scalar.activation(
                        sq, src_t[:, f, :], AF.Square, accum_out=acc[:, f : f + 1]
                    )
                else:
                    sq = tmppool.tile([BPC, D], BF16, tag="tsqd")
                    nc.vector.scalar_tensor_tensor(
                        out=sq,
                        in0=src_t[:, f, :],
                        scalar=1.0,
                        in1=src_t[:, f, :],
                        op0=ALU.mult,
                        op1=ALU.mult,
                        accum_out=acc[:, f : f + 1],
                    )
                tsq_unit += 1
        # 1/sqrt(x) = exp(-0.5*ln(x)): keeps ACT on the natural_log_exp
        # table set (Square/Exp/Ln/Copy coexist there; Sqrt would force a
        # ~1.3us table reload per switch)
        inv_t = small.tile([BPC, FTN], F32)
        nc.scalar.activation(inv_t, nsq_t, AF.Ln)
        nc.scalar.activation(inv_t, inv_t, AF.Exp, scale=-0.5)
        inv_f = small.tile([BPC, FTN], F32)
        nc.scalar.activation(inv_f, nsq_f, AF.Ln)
        nc.scalar.activation(inv_f, inv_f, AF.Exp, scale=-0.5)
        lt = small.tile([BPC, FTN], F32)
        nc.vector.tensor_mul(lt, ltr, inv_t)
        nc.vector.tensor_mul(lt, lt, inv_f)
        nc.vector.tensor_scalar_mul(lt, lt, scale_b[0:BPC, :])
        exp_t = small.tile([BPC, FTN], F32)
        nc.scalar.activation(exp_t, lt, AF.Exp)
        # per-image tail exp sum [32, 1]
        rst = small.tile([BPC, 1], F32)
        nc.vector.tensor_reduce(rst, exp_t, axis=AX.X, op=ALU.add)

        # ---- remaining chunk-0 groups, then chunk 0 post ---------------------
        b0 = GROUPS[0]
        for gsz in GROUPS[1:]:
            do_cgroup(0, b0, gsz)
            b0 += gsz

        # ---- chunk 1 stream; chunk 0 post + partial write hide under it ------
        b0 = 0
        for gi, gsz in enumerate(GROUPS):
            do_cgroup(1, b0, gsz)
            b0 += gsz
            if gi == 0:
                post_chunk(0)
                nc.sync.dma_start(
                    out=part_out[0:1, 0:128].rearrange("o p -> p o"), in_=cs[:, 0:1]
                )
        post_chunk(1)

        lse = small.tile([BPC, 1], F32)
        nc.scalar.activation(lse, rs, AF.Ln)
        dsum = small.tile([128, 1], F32)
        nc.vector.tensor_add(dsum, dcol[:, 0:1], dcol[:, 1:2])

        # u = sum_i lse_i - 2 * sum diag  (single PSUM accumulation)
        u_ps = psum.tile([1, 1], F32, tag="usum")
        nc.tensor.matmul(u_ps, dsum, neg2, start=True, stop=False)
        nc.tensor.matmul(u_ps, lse, ones128[0:BPC, :], start=False, stop=True)
        uv2 = small.tile([1, 1], F32)
        nc.scalar.copy(uv2, u_ps)

        # ---- write out this core's partials (host finishes the loss) --------
        nc.sync.dma_start(
            out=part_out[0:1, 128:256].rearrange("o p -> p o"), in_=cs[:, 1:2]
        )
        nc.sync.dma_start(out=part_out[0:1, 256:257], in_=uv2)

    _cap_sync_waits(nc)
    return nc


_NC = None


def _get_nc() -> bass.Bass:
    global _NC
    if _NC is None:
        _NC = build_nc()
    return _NC


def make_in_maps(inputs: dict) -> list[dict]:
    img_full = np.ascontiguousarray(np.asarray(inputs["image_features"], np.float32))
    rand = np.ascontiguousarray(np.asarray(inputs["random_text_features"], np.float32))
    false = np.asarray(inputs["false_text_features"], np.float32)
    ls = np.asarray(inputs["logit_scale"], np.float32).reshape(1)
    ident = np.eye(128, dtype=np.float32)
    in_maps = []
    for m in range(NCORES):
        sl = slice(m * BPC, (m + 1) * BPC)
        dm = np.zeros((128, 2 * BPC), np.float32)
        a = m * BPC + np.arange(BPC)
        dm[a % 128, (a // 128) * BPC + np.arange(BPC)] = 1.0
        in_maps.append(
            {
                "img": np.ascontiguousarray(img_full[sl]),
                "rand": rand,
                "falset": np.ascontiguousarray(false[m * BPC * FTN : (m + 1) * BPC * FTN]),
                "lscale": ls,
                "ident": ident,
                "dmask": dm,
            }
        )
    return in_maps


def finish_loss(parts: np.ndarray) -> np.ndarray:
    """Combine the 8 per-core [257] partials into the scalar loss.

    parts[m, a<256]: core m's partial column sum of exp(logits) for text a
    parts[m, 256]:   core m's (sum_i lse_i - 2*sum_i diag_i)
    """
    parts = np.asarray(parts, np.float32).reshape(NCORES, 2 * 128 + 1)
    colsum = parts[:, 0:256].sum(axis=0)
    u = parts[:, 256].sum()
    return np.float32((u + np.log(colsum).sum()) / (2.0 * BS)).reshape(())


def kernel(**inputs) -> np.ndarray:
    nc = _get_nc()
    res = run_bass_kernel_spmd(nc, make_in_maps(inputs), list(range(NCORES)))
    parts = np.stack(
        [np.asarray(r["part_out"], np.float32).reshape(-1) for r in res.results]
    )
    return finish_loss(parts)

